# revision 7
# baseline (speedup 1.0000x reference)
"""Trainium2 Bass kernel: block 8x8 2D-DCT + channel-pack + 8x nearest upsample.

Computes, for input x (8, 3, 256, 256) f32:
  out[b, 64c+8a+d, 8i+r, 8j+q] = sum_{m,n} D[a,m] x[b,c,8i+m,8j+n] D[d,n]
i.e. the reference nn_DCT2D: per-8x8-block orthonormal DCT-II, 64 coeffs packed
into channels, then 8x8 nearest-neighbor upsample back to (256, 256).

Strategy (pure data-parallel over batch, one core per batch element):
  - Step 1 (TensorE): A = X^T @ M, the row-DCT over H, where M is the
    block-diagonal DCT factor with columns permuted to c'' = ie*128+8*ip+a
    (i = 2*ip + ie). Output at[kh] [128 x 256] for the two n (image col)
    halves.
  - Step 2 (TensorE): for each (c, ie): psum = A[:, ie-half]^T @ R32[kh],
    where R32 packs ALL 8 output-channel phases d as 32-wide column groups
    (col = 32d + j, j the W-block index). No upsample redundancy in the
    matmul: 8x less PE work than folding the W-replication into the rhs.
  - Fill (DVE/ACT): one broadcast copy per (c, ie, d) expands psum
    [128, 32] -> o4 [128, 2048] with stride-0 dims over r (8x H-rep) and
    q (8x W-rep): f = ie*2048 + r*256 + 8j + q.
  - DMA out (both HWDGE rings): one 2 MB, 128-partition DMA per (c, d) with
    16 KB descriptors: partition (ip, a) -> channel 64c+8a+d rows
    [16ip, 16ip+16), contiguous in HBM.

Everything is f32; matmul accumulation in PSUM f32.
"""

import numpy as np

import concourse.bacc as bacc
import concourse.mybir as mybir
from concourse.tile import TileContext
from concourse.bass_utils import run_bass_kernel_spmd

N_CORES = 8
B, C, H, W = 8, 3, 256, 256
BS = 8          # DCT block size
F32 = mybir.dt.float32


def _dct_matrix() -> np.ndarray:
    n = np.arange(BS, dtype=np.float64)
    k = n[:, None]
    D = np.cos(np.pi * (2.0 * n[None, :] + 1.0) * k / (2.0 * BS))
    scale = np.full((BS,), np.sqrt(2.0 / BS))
    scale[0] = np.sqrt(1.0 / BS)
    return (D * scale[:, None]).astype(np.float32)


def _build_consts() -> tuple[np.ndarray, np.ndarray]:
    D = _dct_matrix()
    # M [2, 128, 256]: col c'' = ie*128 + 8*ip + a maps to DCT row
    # 8*a? no: row k = 8i+m contributes D[a, m] to col c'' with i = 2ip+ie.
    # (ip-major partition order so the output DMA's outer dst dim has 16
    # entries -> descriptors spread over all 16 SDMA engines.)
    Mpp = np.zeros((256, 256), np.float32)
    for k in range(256):
        i = k // 8
        for a in range(8):
            Mpp[k, (i % 2) * 128 + 8 * (i // 2) + a] = D[a, k % 8]
    M = np.stack([Mpp[:128], Mpp[128:]])

    # R32 [2, 128, 256]: R32[kh, n', 32d+j] = D[d, n%8] iff n//8 == j,
    # with n = 128*kh + n'.  (second DCT factor, all 8 d phases packed,
    # j = W-block index 0..31.)
    R32 = np.zeros((2, 128, 256), np.float32)
    for kh in range(2):
        for npr in range(128):
            n = 128 * kh + npr
            j = n // 8
            for d in range(8):
                R32[kh, npr, 32 * d + j] = D[d, n % 8]
    return M, R32


def _build_module():
    nc = bacc.Bacc("TRN2", target_bir_lowering=False, debug=False,
                   enable_asserts=False)

    x_t = nc.dram_tensor("x", [C, H, W], F32, kind="ExternalInput")
    m_t = nc.dram_tensor("m", [2, 128, 256], F32, kind="ExternalInput")
    r_t = nc.dram_tensor("r", [2, 128, 256], F32, kind="ExternalInput")
    out_t = nc.dram_tensor("out", [C * 64, H, W], F32, kind="ExternalOutput")
    # view with channel split into (c, a, d) for the output APs
    out_r = out_t.rearrange("(c a d) h w -> c a d h w", c=C, a=8, d=8)

    with TileContext(nc) as tc:
        with (
            tc.tile_pool(name="consts", bufs=1) as cpool,
            tc.tile_pool(name="xp", bufs=3) as xpool,
            tc.tile_pool(name="atp", bufs=4) as atpool,
            tc.tile_pool(name="outp", bufs=10) as opool,
            tc.tile_pool(name="psa", bufs=2, space="PSUM") as psa_pool,
            tc.tile_pool(name="ps2", bufs=4, space="PSUM") as ps2_pool,
        ):
            # consts: one 256 KB HWDGE DMA each (sync / scalar rings)
            mt = cpool.tile_from(m_t.rearrange("t p f -> p t f"), name="mt",
                                 forced_dma_engine=mybir.EngineType.SP)
            rt = cpool.tile_from(r_t.rearrange("t p f -> p t f"), name="rt",
                                 forced_dma_engine=mybir.EngineType.Activation)

            # images: one 256 KB DMA per channel; c=0 on the fast HWDGE
            # path (needed first), c=1,2 via SWDGE so HWDGE rings stay
            # free for the first output DMAs.
            xt = []
            for c in range(C):
                tile = xpool.tile([128, 2, 256], F32, tag="x", name=f"x{c}")
                src = x_t[c].rearrange("(t p) w -> p t w", t=2)
                eng = nc.sync if c == 0 else nc.gpsimd
                eng.dma_start(out=tile[:, :, :], in_=src)
                xt.append(tile)

            for c in range(C):
                # step 1: at[kh] [n-half, c''=(ie, ip, a)]
                at = []
                for kh in range(2):
                    ps_a = psa_pool.tile([128, 256], F32, tag="psa")
                    for kt in range(2):
                        nc.tensor.matmul(
                            ps_a[:, :],
                            lhsT=xt[c][:, kt, kh * 128:(kh + 1) * 128],
                            rhs=mt[:, kt, :],
                            start=(kt == 0), stop=(kt == 1),
                        )
                    a_sb = atpool.tile([128, 256], F32, tag="at")
                    nc.vector.tensor_copy(out=a_sb[:, :], in_=ps_a[:, :])
                    at.append(a_sb)

                # step 2: both ie halves up front, then per-d fills so each
                # o4 tile completes ~2us after the previous one (the two
                # fill engines work the SAME d concurrently) and its DMA
                # can dispatch immediately -> DMA rings saturate early.
                ps2 = []
                for ie in range(2):
                    ps = ps2_pool.tile([128, 256], F32, tag="ps2",
                                       name=f"ps2_{c}_{ie}")
                    for kh in range(2):
                        nc.tensor.matmul(
                            ps[:, :],
                            lhsT=at[kh][:, ie * 128:(ie + 1) * 128],
                            rhs=rt[:, kh, :],
                            start=(kh == 0), stop=(kh == 1),
                        )
                    ps2.append(ps)

                for d in range(8):
                    o4 = opool.tile([128, 4096], F32, tag="o4",
                                    name=f"o4_{c}_{d}")
                    for ie in range(2):
                        # broadcast copy: [128, 32] -> [128, 8, 32, 8]
                        src_bc = ps2[ie][:, None, d * 32:(d + 1) * 32, None] \
                            .to_broadcast([128, 8, 32, 8])
                        dst = o4[:, ie * 2048:(ie + 1) * 2048].rearrange(
                            "p (r j q) -> p r j q", r=8, q=8)
                        if (d + ie) % 2 == 0:
                            nc.vector.tensor_copy(out=dst, in_=src_bc)
                        else:
                            nc.scalar.copy(out=dst, in_=src_bc)
                    # one 2 MB DMA: partition (ip, a) ->
                    # rows [16ip, 16ip+16) of channel (c, a, d)
                    dstd = out_r[c, :, d].rearrange(
                        "a (ip hh) w -> ip a (hh w)", hh=16)
                    eng = nc.sync if d % 2 == 0 else nc.scalar
                    eng.dma_start(out=dstd, in_=o4[:, :])

    nc.compile()
    return nc


_CACHE: dict = {}


def _get_module():
    if "nc" not in _CACHE:
        _CACHE["nc"] = _build_module()
        _CACHE["consts"] = _build_consts()
    return _CACHE["nc"], _CACHE["consts"]


def kernel(x: np.ndarray) -> np.ndarray:
    x = np.ascontiguousarray(np.asarray(x, dtype=np.float32))
    assert x.shape == (B, C, H, W), x.shape

    nc, (M, R) = _get_module()
    in_maps = [{"x": x[b], "m": M, "r": R} for b in range(N_CORES)]
    res = run_bass_kernel_spmd(nc, in_maps, core_ids=list(range(N_CORES)))
    out = np.stack([res.results[b]["out"] for b in range(N_CORES)], axis=0)
    return out
